# revision 1
# baseline (speedup 1.0000x reference)
"""Trainium2 Bass kernel for nn_CustomAttention (B=8, S=1024, H=1024, NH=16).

Strategy: data-parallel over batch — one batch element per NeuronCore, no
collectives. Host does layout-only prep (transposes for sharding); all FLOPs
run on device.

Per-core dataflow (hsT = hidden_states[b].T, wXT = WX.T):
  QT[o,s] = sum_h wqT[h,o] * hsT[h,s]  (+bq via per-partition tensor_scalar)
  KT[o,s] likewise
  V[s,o]  = sum_h hsT[h,s] * wvT[h,o]  (+bv via K=1 ones-row matmul),
            stored per s-tile as V' [128, NH*65]: per head 64 cols of V plus
            a ones column (col 65) so the ctx matmul also produces the
            softmax denominator (sum over s of exp) in PSUM row 64.
  scoresT[s,l] per head = KT_h(stationary) . QT_h  -> PSUM [128, S]
  expT = Exp(scoresT * 1/sqrt(HD))  (ACT, PSUM->SBUF; no max-subtraction:
         scores ~ N(0,1) so exp is well-conditioned in fp32)
  ctx'T[d,l] (+denom row 64) = V'_h(stationary) . expT  accum over s-tiles
  ctx' drained to SBUF; per 4-head group: denom rows gathered (small DMAs),
  recip = exp(-ln(denom)) batched on ACT (same table set), row DMA'd to a
  partition-0 tile, partition_broadcast (GPSIMD), multiply (DVE), DMA out.

Matmuls run in float32r (full-rate; fp32 is 4 cycles/row). f32r rounds
operands to ~13 mantissa bits at the producer -> end-to-end scale-relative
error ~4e-4 vs the fp32 reference.
"""
import sys

sys.path.insert(0, "/opt/trn_rl_repo")

import numpy as np
from contextlib import ExitStack

from concourse import bacc, tile, mybir
from concourse.bass_utils import run_bass_kernel_spmd

F32 = mybir.dt.float32
F32R = mybir.dt.float32r
AF = mybir.ActivationFunctionType

P = 128
HD = 64
N_CORES = 8


def _chunks(total, size=512):
    out = []
    a = 0
    while a < total:
        out.append((a, min(a + size, total)))
        a += size
    return out


def build_program(S, H, NH, num_devices=N_CORES, reps=1):
    """One SPMD program; every core runs it on its own batch element.

    reps > 1 repeats the whole computation (timing harness only).
    """
    KT = H // P          # h-tiles (contraction tiles)
    NT = H // P          # o-tiles
    ST = S // P          # s-tiles
    HPT = P // HD        # heads per o-tile (2)
    assert NH * HD == H and HPT == 2
    SCALE = 1.0 / float(np.sqrt(HD))

    nc = bacc.Bacc(
        "TRN2", target_bir_lowering=False, debug=False, num_devices=num_devices
    )

    hsT = nc.dram_tensor("hsT", [H, S], F32, kind="ExternalInput")
    wqT = nc.dram_tensor("wqT", [H, H], F32, kind="ExternalInput")
    wkT = nc.dram_tensor("wkT", [H, H], F32, kind="ExternalInput")
    wvT = nc.dram_tensor("wvT", [H, H], F32, kind="ExternalInput")
    bqT = nc.dram_tensor("bqT", [P, NT], F32, kind="ExternalInput")
    bkT = nc.dram_tensor("bkT", [P, NT], F32, kind="ExternalInput")
    # host-provided constants so every matmul operand is DMA-produced --
    # walrus requires f32r matmul inputs to come from f32r-typed producers
    bv_row = nc.dram_tensor("bv_row", [1, H], F32, kind="ExternalInput")
    ones_row = nc.dram_tensor("ones_row", [1, P], F32, kind="ExternalInput")
    onescol = nc.dram_tensor("onescol", [P, NH], F32, kind="ExternalInput")
    outT = nc.dram_tensor("outT", [H, S], F32, kind="ExternalOutput")

    with tile.TileContext(nc) as tc, ExitStack() as ctx:
        consts = ctx.enter_context(tc.tile_pool(name="consts", bufs=1))
        hstp = ctx.enter_context(tc.tile_pool(name="hstp", bufs=KT))
        wstr = ctx.enter_context(tc.tile_pool(name="wstr", bufs=2))
        qtp = ctx.enter_context(tc.tile_pool(name="qtp", bufs=min(4, NT)))
        ktp = ctx.enter_context(tc.tile_pool(name="ktp", bufs=min(4, NT)))
        vvp = ctx.enter_context(tc.tile_pool(name="vvp", bufs=ST))
        big = ctx.enter_context(tc.tile_pool(name="big", bufs=2, space="PSUM"))
        cxp = ctx.enter_context(tc.tile_pool(name="cxp", bufs=2, space="PSUM"))

        # ---- constants ----
        bqT_sb = consts.tile([P, NT], F32, tag="bqT")
        bkT_sb = consts.tile([P, NT], F32, tag="bkT")
        bv_sb = consts.tile([1, H], F32R, tag="bv")
        ones_sb = consts.tile([1, P], F32R, tag="ones")
        nc.sync.dma_start(out=bqT_sb[:], in_=bqT[:])
        nc.sync.dma_start(out=bkT_sb[:], in_=bkT[:])
        nc.sync.dma_start(out=bv_sb[:], in_=bv_row[:].bitcast(F32R))
        nc.sync.dma_start(out=ones_sb[:], in_=ones_row[:].bitcast(F32R))

        pools2 = {}

        for rep in range(reps):
            # ---- load hsT ----
            ht = []
            for k in range(KT):
                t_ = hstp.tile([P, S], F32R, tag="ht", name=f"ht{rep}_{k}")
                nc.sync.dma_start(
                    out=t_[:], in_=hsT[k * P : (k + 1) * P, :].bitcast(F32R)
                )
                ht.append(t_)

            # ---- V' production (wvT streamed through a scoped pool) ----
            vv = []
            assert S == H, "big PSUM pool assumes S == H tile sizes"
            with tc.tile_pool(name=f"wvp{rep}", bufs=KT) as wvp:
                wv = []
                for k in range(KT):
                    t_ = wvp.tile([P, H], F32R, tag="wv", name=f"wv{rep}_{k}")
                    nc.sync.dma_start(
                        out=t_[:], in_=wvT[k * P : (k + 1) * P, :].bitcast(F32R)
                    )
                    wv.append(t_)

                for m in range(ST):
                    ps = big.tile([P, H], F32, tag="big", name=f"vps{rep}_{m}")
                    for (a, b) in _chunks(H):
                        nc.tensor.matmul(
                            ps[:, a:b], ones_sb[:], bv_sb[:, a:b],
                            start=True, stop=False,
                        )
                    for k in range(KT):
                        lhs = ht[k][:, m * P : (m + 1) * P]
                        for (a, b) in _chunks(H):
                            nc.tensor.matmul(
                                ps[:, a:b], lhs, wv[k][:, a:b],
                                start=False, stop=(k == KT - 1),
                            )
                    vt = vvp.tile(
                        [P, NH * 65], F32R, tag="vv", name=f"vv{rep}_{m}"
                    )
                    vview = vt[:].rearrange("p (h e) -> p h e", e=65)
                    nc.vector.tensor_copy(
                        vview[:, :, 0:64],
                        ps[:].rearrange("p (h d) -> p h d", d=HD),
                    )
                    nc.sync.dma_start(
                        out=vview[:, :, 64:65], in_=onescol[:].bitcast(F32R)
                    )
                    vv.append(vt)

            if not pools2:
                pools2["exp_pool"] = ctx.enter_context(
                    tc.tile_pool(name="exp_pool", bufs=5)
                )
                pools2["cup"] = ctx.enter_context(tc.tile_pool(name="cup", bufs=8))
                pools2["denp"] = ctx.enter_context(tc.tile_pool(name="denp", bufs=2))
                pools2["bsp"] = ctx.enter_context(tc.tile_pool(name="bsp", bufs=2))
                pools2["bcp"] = ctx.enter_context(tc.tile_pool(name="bcp", bufs=2))
                pools2["outp"] = ctx.enter_context(tc.tile_pool(name="outp", bufs=3))
            exp_pool = pools2["exp_pool"]
            cup = pools2["cup"]
            denp = pools2["denp"]
            bsp = pools2["bsp"]
            bcp = pools2["bcp"]
            outp = pools2["outp"]

            # ---- per o-tile: QT/KT projection then attention for its heads --
            group_cu = []  # drained ctx' tiles of the current 4-head group
            for t in range(NT):
                proj_out = []
                for (wT, bias_sb, pool, tag) in (
                    (wqT, bqT_sb, qtp, "qt"),
                    (wkT, bkT_sb, ktp, "kt"),
                ):
                    wtile = wstr.tile(
                        [P, KT, P], F32R, tag="wstr", name=f"w{tag}{rep}_{t}"
                    )
                    nc.sync.dma_start(
                        out=wtile[:],
                        in_=wT[:, t * P : (t + 1) * P]
                        .rearrange("(k p) c -> p k c", p=P)
                        .bitcast(F32R),
                    )
                    ps = big.tile([P, S], F32, tag="big", name=f"pps{rep}_{t}{tag}")
                    for k in range(KT):
                        for (a, b) in _chunks(S):
                            nc.tensor.matmul(
                                ps[:, a:b], wtile[:, k, :], ht[k][:, a:b],
                                start=(k == 0), stop=(k == KT - 1),
                            )
                    ot = pool.tile([P, S], F32R, tag=tag, name=f"{tag}{rep}_{t}")
                    nc.vector.tensor_scalar_add(
                        ot[:], ps[:], bias_sb[:, t : t + 1]
                    )
                    proj_out.append(ot)
                qt_t, kt_t = proj_out

                # heads 2t (rows 0:64) and 2t+1 (rows 64:128)
                cx = [
                    cxp.tile([65, S], F32, tag="cx", name=f"cx{rep}_{t}_{i}")
                    for i in range(HPT)
                ]
                for j in range(ST):
                    for hh in range(HPT):
                        r0, r1 = hh * HD, (hh + 1) * HD
                        sc = big.tile(
                            [P, S], F32, tag="big", name=f"sc{rep}_{t}_{j}_{hh}"
                        )
                        for (a, b) in _chunks(S):
                            nc.tensor.matmul(
                                sc[:, a:b],
                                kt_t[r0:r1, j * P : (j + 1) * P],
                                qt_t[r0:r1, a:b],
                                start=True, stop=True,
                                tile_position=(r0, 0),
                            )
                        ex = exp_pool.tile(
                            [P, S], F32R, tag="ex", name=f"ex{rep}_{t}_{j}_{hh}"
                        )
                        nc.scalar.activation(ex[:], sc[:], AF.Exp, scale=SCALE)
                        h = HPT * t + hh
                        lhs = vv[j][:, h * 65 : (h + 1) * 65]
                        for (a, b) in _chunks(S):
                            nc.tensor.matmul(
                                cx[hh][0:65, a:b], lhs, ex[:, a:b],
                                start=(j == 0), stop=(j == ST - 1),
                            )

                # drain ctx' to SBUF (frees PSUM quickly)
                for hh in range(HPT):
                    cu = cup.tile(
                        [65, S], F32, tag="cu", name=f"cu{rep}_{t}_{hh}"
                    )
                    nc.vector.tensor_copy(cu[:], cx[hh][:])
                    group_cu.append((HPT * t + hh, cu))

                # normalization for each 4-head group (2 o-tiles)
                if t % 2 == 1:
                    g = len(group_cu)
                    den = denp.tile([g, S], F32, tag="den", name=f"den{rep}_{t}")
                    for i, (h, cu) in enumerate(group_cu):
                        nc.sync.dma_start(
                            out=den[i : i + 1, :], in_=cu[64:65, :]
                        )
                    nc.scalar.activation(den[:], den[:], AF.Ln)
                    nc.scalar.activation(den[:], den[:], AF.Exp, scale=-1.0)
                    for i, (h, cu) in enumerate(group_cu):
                        bsrc = bsp.tile([1, S], F32, tag="bsrc", name=f"bs{rep}_{h}")
                        nc.sync.dma_start(out=bsrc[:], in_=den[i : i + 1, :])
                        bc = bcp.tile([HD, S], F32, tag="bc", name=f"bc{rep}_{h}")
                        nc.gpsimd.partition_broadcast(bc[:], bsrc[:])
                        ou = outp.tile([HD, S], F32, tag="ou", name=f"ou{rep}_{h}")
                        nc.vector.tensor_mul(ou[:], cu[0:64, :], bc[:])
                        nc.sync.dma_start(
                            out=outT[h * HD : (h + 1) * HD, :], in_=ou[:]
                        )
                    group_cu = []

    nc.compile()
    return nc


_CACHE = {}


def _get_program(S, H, NH, num_devices):
    key = (S, H, NH, num_devices)
    if key not in _CACHE:
        _CACHE[key] = build_program(S, H, NH, num_devices)
    return _CACHE[key]


def make_in_maps(hidden_states, Wq, bq, Wk, bk, Wv, bv):
    B, S, H = hidden_states.shape
    NH = H // HD
    NT = H // P
    wqT = np.ascontiguousarray(Wq.T.astype(np.float32))
    wkT = np.ascontiguousarray(Wk.T.astype(np.float32))
    wvT = np.ascontiguousarray(Wv.T.astype(np.float32))
    bqT = np.ascontiguousarray(bq.reshape(NT, P).T.astype(np.float32))
    bkT = np.ascontiguousarray(bk.reshape(NT, P).T.astype(np.float32))
    bv_row = bv.astype(np.float32).reshape(1, H)
    ones_row = np.ones((1, P), np.float32)
    ones_col = np.ones((P, NH), np.float32)
    in_maps = []
    for b in range(B):
        in_maps.append(
            {
                "hsT": np.ascontiguousarray(hidden_states[b].T.astype(np.float32)),
                "wqT": wqT,
                "wkT": wkT,
                "wvT": wvT,
                "bqT": bqT,
                "bkT": bkT,
                "bv_row": bv_row,
                "ones_row": ones_row,
                "onescol": ones_col,
            }
        )
    return in_maps


def kernel(hidden_states, Wq, bq, Wk, bk, Wv, bv):
    hidden_states = np.asarray(hidden_states, dtype=np.float32)
    Wq = np.asarray(Wq, dtype=np.float32)
    bq = np.asarray(bq, dtype=np.float32)
    Wk = np.asarray(Wk, dtype=np.float32)
    bk = np.asarray(bk, dtype=np.float32)
    Wv = np.asarray(Wv, dtype=np.float32)
    bv = np.asarray(bv, dtype=np.float32)

    B, S, H = hidden_states.shape
    NH = H // HD
    assert B == N_CORES, "one batch element per core"

    nc = _get_program(S, H, NH, N_CORES)
    in_maps = make_in_maps(hidden_states, Wq, bq, Wk, bk, Wv, bv)
    res = run_bass_kernel_spmd(nc, in_maps, core_ids=list(range(N_CORES)))
    out = np.empty((B, S, H), np.float32)
    for b in range(B):
        out[b] = res.results[b]["outT"].T
    return out


if __name__ == "__main__":
    build_program(1024, 1024, 16)
    print("build ok")



# revision 14
# speedup vs baseline: 1.5494x; 1.5494x over previous
"""Trainium2 Bass kernel for nn_CustomAttention (B=8, S=1024, H=1024, NH=16).

Strategy: data-parallel over batch — one batch element per NeuronCore, no
collectives. Host does layout-only prep (transposes / bf16 casts); all FLOPs
run on device.

Per-core dataflow (hsT = hidden_states[b].T in bf16, weights pre-transposed
and pre-tiled to bf16 on host):
  QT[o,l] = sum_h wqT[h,o] * hsT[h,l]   (f32 PSUM; +bq via per-partition
  KT[o,l] likewise                       tensor_scalar_add -> f32r SBUF)
  V[s,o]  = sum_h hsT[h,s] * wvT[h,o]   -> per s-tile V' [128, NH, 66] bf16:
            per head 64 cols of V plus a ones column (col 64) used as the
            moving ones-vector that produces softmax denominators.
  scoresT[s,l] per head = KT_h(stationary) . QT_h -> PSUM [128, S]
  expT = Exp(scoresT * 1/sqrt(HD)) -> bf16 SBUF (no max-subtraction: scores
         ~ N(0,1) so exp is well-conditioned)
  ctx[l,d] per (head, l-tile): stationary = expT chunk [s,128], moving =
         V'_h [s, 65] (64 V cols + ones col) accumulated over s-tiles in a
         one-bank PSUM tile [128, 130 used of 512]: cols 0:64 h0-ctx,
         64:65 h0-den, 65:129 h1-ctx, 129:130 h1-den.
  normalize: DVE reciprocal of den cols, then scalar_tensor_tensor
         out = ctx * recip + bv  (bv folded in here: sum(probs)=1).
  out[l, o-tile] tiles gather into a per-o-tile [128, ST, 128] staging tile,
  one DMA per o-tile into out[S, H].

Schedule: software-pipelined sections. Section t runs scores+exp(t),
ctx+normalize(t-2), and smears proj(t+1) k-steps plus (sections 0-1) the V'
production between the Act-paced scores so the PE never waits on the
exp->PSUM-free chain. PSUM: scores 2x2 banks, proj 1x2, ctx 2x1 = 8 banks.

Matmul cost on TRN2 is (moving free size) x cycles/row with stationary loads
free, so the transposed ctx (N=65 per 128x128 stationary) halves ctx cost vs
streaming expT as the moving operand. bf16 keeps full matmul rate and halves
DMA; end-to-end error vs the fp32 reference ~4e-3 (tolerance 2e-2).
"""
import sys

sys.path.insert(0, "/opt/trn_rl_repo")

import numpy as np
import ml_dtypes
from contextlib import ExitStack

from concourse import bacc, tile, mybir
from concourse.bass_utils import run_bass_kernel_spmd

F32 = mybir.dt.float32
F32R = mybir.dt.float32r
BF16 = mybir.dt.bfloat16
AF = mybir.ActivationFunctionType
ALU = mybir.AluOpType

P = 128
HD = 64
N_CORES = 8


def _chunks(total, size=512):
    out = []
    a = 0
    while a < total:
        out.append((a, min(a + size, total)))
        a += size
    return out


def build_program(S, H, NH, num_devices=N_CORES):
    """One SPMD program; every core runs it on its own batch element."""
    KT = H // P          # h-tiles (contraction tiles)
    NT = H // P          # o-tiles
    ST = S // P          # s-tiles / l-tiles
    HPT = P // HD        # heads per o-tile (2)
    assert NH * HD == H and HPT == 2 and S == H
    SCALE = 1.0 / float(np.sqrt(HD))

    nc = bacc.Bacc(
        "TRN2", target_bir_lowering=False, debug=False, num_devices=num_devices
    )

    hsT = nc.dram_tensor("hsT", [H, S], BF16, kind="ExternalInput")
    # wq/wk pre-tiled on host: row (t*P+p) = concat_k wqT[k*P+p, t*P:(t+1)*P]
    wqTp = nc.dram_tensor("wqTp", [NT * P, KT * P], BF16, kind="ExternalInput")
    wkTp = nc.dram_tensor("wkTp", [NT * P, KT * P], BF16, kind="ExternalInput")
    wvT = nc.dram_tensor("wvT", [H, H], BF16, kind="ExternalInput")
    bqk = nc.dram_tensor("bqk", [P, 2 * NT], F32, kind="ExternalInput")
    bv_row = nc.dram_tensor("bv_row", [1, H], F32, kind="ExternalInput")
    outD = nc.dram_tensor("out", [S, H], F32, kind="ExternalOutput")

    with tile.TileContext(nc) as tc, ExitStack() as ctx:
        consts = ctx.enter_context(tc.tile_pool(name="consts", bufs=1))
        hstp = ctx.enter_context(tc.tile_pool(name="hstp", bufs=KT))
        wstr = ctx.enter_context(tc.tile_pool(name="wstr", bufs=4))
        qkp = ctx.enter_context(tc.tile_pool(name="qkp", bufs=4))
        vvp = ctx.enter_context(tc.tile_pool(name="vvp", bufs=ST))
        expp = ctx.enter_context(tc.tile_pool(name="expp", bufs=6 * ST))
        rpl = ctx.enter_context(tc.tile_pool(name="rpl", bufs=4))
        outp = ctx.enter_context(tc.tile_pool(name="outp", bufs=3))
        big = ctx.enter_context(tc.tile_pool(name="big", bufs=2, space="PSUM"))
        prp = ctx.enter_context(tc.tile_pool(name="prp", bufs=1, space="PSUM"))
        cxp = ctx.enter_context(tc.tile_pool(name="cxp", bufs=2, space="PSUM"))

        # ---- input DMA: w(0) and first hsT tiles first (feed the PE asap) --
        wq_t = {}
        wk_t = {}

        def load_w(t):
            if t >= NT or t in wq_t:
                return
            for name, dram, store in (("wq", wqTp, wq_t), ("wk", wkTp, wk_t)):
                w = wstr.tile([P, KT, P], BF16, tag="wstr", name=f"{name}{t}")
                nc.sync.dma_start(
                    out=w[:],
                    in_=dram[t * P : (t + 1) * P, :].rearrange(
                        "p (k c) -> p k c", c=P
                    ),
                )
                store[t] = w

        load_w(0)

        # q/k biases: one small DMA, needed first at the proj(0) bias-add
        consts_bqk = consts.tile([P, 2 * NT], F32, tag="bqk")
        nc.sync.dma_start(out=consts_bqk[:], in_=bqk[:])

        ht = []
        for k in range(KT):
            t_ = hstp.tile([P, S], BF16, tag="ht", name=f"ht{k}")
            nc.sync.dma_start(out=t_[:], in_=hsT[k * P : (k + 1) * P, :])
            ht.append(t_)

        load_w(1)

        wvp = ctx.enter_context(tc.tile_pool(name="wvp", bufs=KT))
        wv = []
        for k in range(KT):
            t_ = wvp.tile([P, H], BF16, tag="wv", name=f"wv{k}")
            nc.sync.dma_start(out=t_[:], in_=wvT[k * P : (k + 1) * P, :])
            wv.append(t_)

        # ---- bv broadcast (first needed at the first ctx normalize) ----
        bv_sb = consts.tile([1, H], F32, tag="bv")
        bvb = consts.tile([P, H], F32, tag="bvb")
        nc.sync.dma_start(out=bv_sb[:], in_=bv_row[:])
        nc.gpsimd.partition_broadcast(bvb[:], bv_sb[:])

        qt_t = {}
        kt_t = {}
        ex_t = {}  # t -> {(hh, j): exp tile}
        vv = []

        # ---- emission helpers ----
        def proj_steps(t):
            """Generator yielding k-step closures for Q then K of o-tile t;
            accumulates into the dedicated proj psum, drains via DVE."""
            for w, bcol, store, tag in (
                (wq_t[t], t, qt_t, "qt"),
                (wk_t[t], NT + t, kt_t, "kt"),
            ):
                ps = prp.tile([P, S], F32, tag="pr", name=f"pps{t}{tag}")
                for k in range(KT):
                    for (a, b) in _chunks(S):
                        nc.tensor.matmul(
                            ps[:, a:b], w[:, k, :], ht[k][:, a:b],
                            start=(k == 0), stop=(k == KT - 1),
                        )
                    yield
                ot = qkp.tile([P, S], F32R, tag=tag, name=f"{tag}{t}")
                nc.vector.tensor_scalar_add(
                    ot[:], ps[:], consts_bqk[:, bcol : bcol + 1]
                )
                store[t] = ot
            while True:
                yield

        def vprime_steps():
            """Generator yielding one V' m-tile per step."""
            for m in range(ST):
                ps = big.tile([P, S], F32, tag="big", name=f"vps{m}")
                for k in range(KT):
                    lhs = ht[k][:, m * P : (m + 1) * P]
                    for (a, b) in _chunks(H):
                        nc.tensor.matmul(
                            ps[:, a:b], lhs, wv[k][:, a:b],
                            start=(k == 0), stop=(k == KT - 1),
                        )
                vt = vvp.tile([P, NH, 66], BF16, tag="vv", name=f"vv{m}")
                nc.vector.tensor_copy(
                    vt[:, :, 0:64], ps[:].rearrange("p (h d) -> p h d", d=HD)
                )
                nc.vector.memset(vt[:, :, 64:65], 1.0)
                vv.append(vt)
                yield
            while True:
                yield

        def emit_scores_exp(t, j):
            exs = ex_t.setdefault(t, {})
            for hh in range(HPT):
                r0 = hh * HD
                sc = big.tile([P, S], F32, tag="big", name=f"sc{t}_{j}_{hh}")
                for (a, b) in _chunks(S):
                    nc.tensor.matmul(
                        sc[:, a:b],
                        kt_t[t][r0 : r0 + HD, j * P : (j + 1) * P],
                        qt_t[t][r0 : r0 + HD, a:b],
                        start=True, stop=True,
                        tile_position=(r0, 0),
                    )
                e = expp.tile([P, S], BF16, tag="ex", name=f"ex{t}_{j}_{hh}")
                nc.scalar.activation(e[:], sc[:], AF.Exp, scale=SCALE)
                exs[(hh, j)] = e

        def emit_ctx(t, l, ot):
            exs = ex_t[t]
            cx = cxp.tile([P, 512], F32, tag="cx", name=f"cx{t}_{l}")
            for hh in range(HPT):
                h = HPT * t + hh
                o = hh * 65
                for j in range(ST):
                    nc.tensor.matmul(
                        cx[:, o : o + 65],
                        exs[(hh, j)][:, l * P : (l + 1) * P],
                        vv[j][:, h, 0:65],
                        start=(j == 0), stop=(j == ST - 1),
                    )
            rc = rpl.tile([P, HPT], F32, tag="rc", name=f"rc{t}_{l}")
            dens = cx[:, 0 : 2 * 65].rearrange("p (h x) -> p h x", x=65)[:, :, 64:65]
            nc.vector.reciprocal(rc[:].rearrange("p (h x) -> p h x", x=1), dens)
            for hh in range(HPT):
                nc.vector.scalar_tensor_tensor(
                    ot[:, l, hh * HD : (hh + 1) * HD],
                    cx[:, hh * 65 : hh * 65 + 64],
                    rc[:, hh : hh + 1],
                    bvb[:, (HPT * t + hh) * HD : (HPT * t + hh + 1) * HD],
                    ALU.mult,
                    ALU.add,
                )

        # ---- software-pipelined schedule ----
        # preamble: proj(0) unsmeared (Act idle anyway at start)
        p0 = proj_steps(0)
        for _ in range(2 * KT + 1):
            next(p0)

        vgen = vprime_steps()
        nvp = 0  # V' tiles emitted so far

        ots = {}
        ctx_done = {}

        def ctx_unit(tc_, l, split_dma=False):
            """Emit one ctx+normalize unit; DMA the o-tile column when all
            ST units of tc_ have been emitted (split_dma: one DMA per l)."""
            if tc_ not in ots:
                ots[tc_] = outp.tile([P, ST, P], F32, tag="ou", name=f"ou{tc_}")
                ctx_done[tc_] = 0
            emit_ctx(tc_, l, ots[tc_])
            ctx_done[tc_] += 1
            if split_dma:
                nc.sync.dma_start(
                    out=outD[l * P : (l + 1) * P, tc_ * P : (tc_ + 1) * P],
                    in_=ots[tc_][:, l, :],
                )
            elif ctx_done[tc_] == ST:
                nc.sync.dma_start(
                    out=outD[:, tc_ * P : (tc_ + 1) * P].rearrange(
                        "(l p) c -> p l c", p=P
                    ),
                    in_=ots[tc_][:],
                )

        # sections 0..NT-1: scores/exp(t), ctx(t-2), proj(t+1) smear, V' smear
        for t in range(NT):
            pgen = proj_steps(t + 1) if t + 1 < NT else None
            for j in range(ST):
                emit_scores_exp(t, j)
                # V' smear: 4 tiles in section 0 (after wv DMAs land), 4 in 1
                if nvp < ST and (t == 0 and j >= 3 or t == 1):
                    next(vgen)
                    nvp += 1
                if t >= 2:
                    ctx_unit(t - 2, j)
                if t == NT - 1 and j >= 2:
                    ctx_unit(NT - 2, j - 2)  # pull-in: no proj smear this section
                if pgen is not None:
                    next(pgen)  # 2 k-steps per j
                    next(pgen)
            if pgen is not None:
                next(pgen)  # flush the trailing bias-add
            load_w(t + 2)

        # tail: remaining ctx(NT-2) units, then ctx(NT-1) with per-l DMAs
        for l in (ST - 2, ST - 1):
            ctx_unit(NT - 2, l)
        for l in range(ST):
            ctx_unit(NT - 1, l, split_dma=True)

    nc.compile()
    return nc


_CACHE = {}


def _get_program(S, H, NH, num_devices):
    key = (S, H, NH, num_devices)
    if key not in _CACHE:
        _CACHE[key] = build_program(S, H, NH, num_devices)
    return _CACHE[key]


def make_in_maps(hidden_states, Wq, bq, Wk, bk, Wv, bv):
    B, S, H = hidden_states.shape
    NT = H // P
    KT = H // P
    # wq/wk pre-tiled: row (t*P+p) holds concat over k of wT[k*P+p, t*P:(t+1)*P]
    def pack_w(W):
        wT = np.ascontiguousarray(W.T.astype(np.float32))  # [h, o]
        w4 = wT.reshape(KT, P, NT, P)                      # [k, p, t, c]
        return np.ascontiguousarray(
            w4.transpose(2, 1, 0, 3).reshape(NT * P, KT * P)
        ).astype(ml_dtypes.bfloat16)

    wqTp = pack_w(Wq)
    wkTp = pack_w(Wk)
    wvT = np.ascontiguousarray(Wv.T.astype(np.float32)).astype(ml_dtypes.bfloat16)
    bqk = np.ascontiguousarray(
        np.concatenate(
            [bq.reshape(NT, P).T, bk.reshape(NT, P).T], axis=1
        ).astype(np.float32)
    )
    bv_row = bv.astype(np.float32).reshape(1, H)
    in_maps = []
    for b in range(B):
        in_maps.append(
            {
                "hsT": np.ascontiguousarray(
                    hidden_states[b].T.astype(np.float32)
                ).astype(ml_dtypes.bfloat16),
                "wqTp": wqTp,
                "wkTp": wkTp,
                "wvT": wvT,
                "bqk": bqk,
                "bv_row": bv_row,
            }
        )
    return in_maps


def kernel(hidden_states, Wq, bq, Wk, bk, Wv, bv):
    hidden_states = np.asarray(hidden_states, dtype=np.float32)
    Wq = np.asarray(Wq, dtype=np.float32)
    bq = np.asarray(bq, dtype=np.float32)
    Wk = np.asarray(Wk, dtype=np.float32)
    bk = np.asarray(bk, dtype=np.float32)
    Wv = np.asarray(Wv, dtype=np.float32)
    bv = np.asarray(bv, dtype=np.float32)

    B, S, H = hidden_states.shape
    NH = H // HD
    assert B == N_CORES, "one batch element per core"

    nc = _get_program(S, H, NH, N_CORES)
    in_maps = make_in_maps(hidden_states, Wq, bq, Wk, bk, Wv, bv)
    res = run_bass_kernel_spmd(nc, in_maps, core_ids=list(range(N_CORES)))
    out = np.empty((B, S, H), np.float32)
    for b in range(B):
        out[b] = res.results[b]["out"]
    return out


if __name__ == "__main__":
    build_program(1024, 1024, 16)
    print("build ok")


# revision 27
# speedup vs baseline: 1.5836x; 1.0221x over previous
"""Trainium2 Bass kernel for nn_CustomAttention (B=8, S=1024, H=1024, NH=16).

Strategy: data-parallel over batch — one batch element per NeuronCore, no
collectives. Host does layout-only prep (transposes / bf16 casts); all FLOPs
run on device.

Per-core dataflow (hsT = hidden_states[b].T in bf16, weights pre-transposed
and pre-tiled to bf16 on host):
  QT[o,l] = sum_h wqT[h,o] * hsT[h,l]   (f32 PSUM; +bq via per-partition
  KT[o,l] likewise                       tensor_scalar_add -> f32r SBUF)
  V[s,o]  = sum_h hsT[h,s] * wvT[h,o]   -> per s-tile V' [128, NH, 66] bf16:
            per head 64 cols of V plus a ones column (col 64) used as the
            moving ones-vector that produces softmax denominators.
  scoresT[s,l] per head = KT_h(stationary) . QT_h -> PSUM [128, S]
  expT = Exp(scoresT * 1/sqrt(HD)) -> bf16 SBUF (no max-subtraction: scores
         ~ N(0,1) so exp is well-conditioned)
  ctx[l,d] per (head, l-tile): stationary = expT chunk [s,128], moving =
         V'_h [s, 65] (64 V cols + ones col) accumulated over s-tiles in a
         one-bank PSUM tile [128, 130 used of 512]: cols 0:64 h0-ctx,
         64:65 h0-den, 65:129 h1-ctx, 129:130 h1-den.
  normalize: DVE reciprocal of den cols, then scalar_tensor_tensor
         out = ctx * recip + bv  (bv folded in here: sum(probs)=1).
  out[l, o-tile] tiles gather into a per-o-tile [128, ST, 128] staging tile,
  one DMA per o-tile into out[S, H].

Schedule: software-pipelined sections. Section t runs scores+exp(t),
ctx+normalize(t-2), and smears proj(t+1) k-steps plus (sections 0-1) the V'
production between the Act-paced scores so the PE never waits on the
exp->PSUM-free chain. PSUM: scores 2x2 banks, proj 1x2, ctx 2x1 = 8 banks.

Matmul cost on TRN2 is (moving free size) x cycles/row with stationary loads
free, so the transposed ctx (N=65 per 128x128 stationary) halves ctx cost vs
streaming expT as the moving operand. bf16 keeps full matmul rate and halves
DMA; end-to-end error vs the fp32 reference ~4e-3 (tolerance 2e-2).
"""
import sys

sys.path.insert(0, "/opt/trn_rl_repo")

import numpy as np
import ml_dtypes
from contextlib import ExitStack

from concourse import bacc, tile, mybir
from concourse.bass_utils import run_bass_kernel_spmd

F32 = mybir.dt.float32
F32R = mybir.dt.float32r
BF16 = mybir.dt.bfloat16
AF = mybir.ActivationFunctionType
ALU = mybir.AluOpType

P = 128
HD = 64
N_CORES = 8


def _chunks(total, size=512):
    out = []
    a = 0
    while a < total:
        out.append((a, min(a + size, total)))
        a += size
    return out


def build_program(S, H, NH, num_devices=N_CORES):
    """One SPMD program; every core runs it on its own batch element."""
    KT = H // P          # h-tiles (contraction tiles)
    NT = H // P          # o-tiles
    ST = S // P          # s-tiles / l-tiles
    HPT = P // HD        # heads per o-tile (2)
    assert NH * HD == H and HPT == 2 and S == H
    SCALE = 1.0 / float(np.sqrt(HD))

    nc = bacc.Bacc(
        "TRN2", target_bir_lowering=False, debug=False, num_devices=num_devices
    )

    hsT = nc.dram_tensor("hsT", [H, S], BF16, kind="ExternalInput")
    # wq/wk pre-tiled on host: row (t*P+p) = concat_k wqT[k*P+p, t*P:(t+1)*P]
    wqTp = nc.dram_tensor("wqTp", [NT * P, KT * P], BF16, kind="ExternalInput")
    wkTp = nc.dram_tensor("wkTp", [NT * P, KT * P], BF16, kind="ExternalInput")
    wvT = nc.dram_tensor("wvT", [H, H], BF16, kind="ExternalInput")
    bqk = nc.dram_tensor("bqk", [P, 2 * NT], F32, kind="ExternalInput")
    bv_row = nc.dram_tensor("bv_row", [1, H], F32, kind="ExternalInput")
    outD = nc.dram_tensor("out", [S, H], F32, kind="ExternalOutput")

    with tile.TileContext(nc) as tc, ExitStack() as ctx:
        consts = ctx.enter_context(tc.tile_pool(name="consts", bufs=1))
        hstp = ctx.enter_context(tc.tile_pool(name="hstp", bufs=KT))
        wstr = ctx.enter_context(tc.tile_pool(name="wstr", bufs=4))
        qkp = ctx.enter_context(tc.tile_pool(name="qkp", bufs=4))
        vvp = ctx.enter_context(tc.tile_pool(name="vvp", bufs=ST))
        expp = ctx.enter_context(tc.tile_pool(name="expp", bufs=6 * ST))
        rpl = ctx.enter_context(tc.tile_pool(name="rpl", bufs=4))
        outp = ctx.enter_context(tc.tile_pool(name="outp", bufs=3))
        big = ctx.enter_context(tc.tile_pool(name="big", bufs=2, space="PSUM"))
        prp = ctx.enter_context(tc.tile_pool(name="prp", bufs=1, space="PSUM"))
        cxp = ctx.enter_context(tc.tile_pool(name="cxp", bufs=2, space="PSUM"))

        # ---- input DMA: w(0) and first hsT tiles first (feed the PE asap) --
        wq_t = {}
        wk_t = {}

        def load_w(t):
            if t >= NT or t in wq_t:
                return
            for name, dram, store in (("wq", wqTp, wq_t), ("wk", wkTp, wk_t)):
                w = wstr.tile([P, KT, P], BF16, tag="wstr", name=f"{name}{t}")
                nc.sync.dma_start(
                    out=w[:],
                    in_=dram[t * P : (t + 1) * P, :].rearrange(
                        "p (k c) -> p k c", c=P
                    ),
                )
                store[t] = w

        load_w(0)

        # hsT tiles ride the (otherwise idle until ~10us) DVE/Act DMA queues
        # so they land in parallel with the SP-queue weight loads.
        ht = []
        for k in range(KT):
            t_ = hstp.tile([P, S], BF16, tag="ht", name=f"ht{k}")
            eng = nc.gpsimd if k % 2 == 0 else nc.scalar
            eng.dma_start(out=t_[:], in_=hsT[k * P : (k + 1) * P, :])
            ht.append(t_)

        # q/k biases: one small DMA, needed first at the proj(0) bias-add
        consts_bqk = consts.tile([P, 2 * NT], F32, tag="bqk")
        nc.sync.dma_start(out=consts_bqk[:], in_=bqk[:])

        wvp = ctx.enter_context(tc.tile_pool(name="wvp", bufs=KT))
        wv = []
        for k in range(KT):
            t_ = wvp.tile([P, H], BF16, tag="wv", name=f"wv{k}")
            nc.sync.dma_start(out=t_[:], in_=wvT[k * P : (k + 1) * P, :])
            wv.append(t_)

        load_w(1)

        # ---- bv broadcast (first needed at the first ctx normalize) ----
        bv_sb = consts.tile([1, H], F32, tag="bv")
        bvb = consts.tile([P, H], F32, tag="bvb")
        nc.sync.dma_start(out=bv_sb[:], in_=bv_row[:])
        nc.gpsimd.partition_broadcast(bvb[:], bv_sb[:])

        qt_t = {}
        kt_t = {}
        ex_t = {}  # t -> {(hh, j): exp tile}
        vv = []

        # ---- emission helpers ----
        def proj_steps(t, kpool=None):
            """Generator yielding k-step closures for Q then K of o-tile t;
            accumulates into the dedicated proj psum, drains via DVE.
            kpool: alternate pool for the K projection (preamble only)."""
            for w, bcol, store, tag, pool_, ptag in (
                (wq_t[t], t, qt_t, "qt", prp, "pr"),
                (wk_t[t], NT + t, kt_t, "kt", kpool or prp,
                 "big" if kpool is not None else "pr"),
            ):
                ps = pool_.tile([P, S], F32, tag=ptag, name=f"pps{t}{tag}")
                for k in range(KT):
                    for (a, b) in _chunks(S):
                        nc.tensor.matmul(
                            ps[:, a:b], w[:, k, :], ht[k][:, a:b],
                            start=(k == 0), stop=(k == KT - 1),
                        )
                    yield
                ot = qkp.tile([P, S], F32R, tag=tag, name=f"{tag}{t}")
                nc.vector.tensor_scalar_add(
                    ot[:], ps[:], consts_bqk[:, bcol : bcol + 1]
                )
                store[t] = ot
            while True:
                yield

        def vprime_steps():
            """Generator yielding one V' m-tile per step."""
            for m in range(ST):
                ps = big.tile([P, S], F32, tag="big", name=f"vps{m}")
                for k in range(KT):
                    lhs = ht[k][:, m * P : (m + 1) * P]
                    for (a, b) in _chunks(H):
                        nc.tensor.matmul(
                            ps[:, a:b], lhs, wv[k][:, a:b],
                            start=(k == 0), stop=(k == KT - 1),
                        )
                vt = vvp.tile([P, NH, 66], BF16, tag="vv", name=f"vv{m}")
                nc.vector.tensor_copy(
                    vt[:, :, 0:64], ps[:].rearrange("p (h d) -> p h d", d=HD)
                )
                nc.vector.memset(vt[:, :, 64:65], 1.0)
                vv.append(vt)
                yield
            while True:
                yield

        def emit_scores_exp(t, j):
            exs = ex_t.setdefault(t, {})
            for hh in range(HPT):
                r0 = hh * HD
                sc = big.tile([P, S], F32, tag="big", name=f"sc{t}_{j}_{hh}")
                for (a, b) in _chunks(S):
                    nc.tensor.matmul(
                        sc[:, a:b],
                        kt_t[t][r0 : r0 + HD, j * P : (j + 1) * P],
                        qt_t[t][r0 : r0 + HD, a:b],
                        start=True, stop=True,
                        tile_position=(r0, 0),
                    )
                e = expp.tile([P, S], BF16, tag="ex", name=f"ex{t}_{j}_{hh}")
                nc.scalar.activation(e[:], sc[:], AF.Exp, scale=SCALE)
                exs[(hh, j)] = e

        def emit_ctx(t, l, ot, norm_eng=None, cx_pool=None, cx_tag="cx"):
            exs = ex_t[t]
            pool_ = cx_pool or cxp
            cx = pool_.tile([P, 512], F32, tag=cx_tag, name=f"cx{t}_{l}")
            for hh in range(HPT):
                h = HPT * t + hh
                o = hh * 65
                for j in range(ST):
                    nc.tensor.matmul(
                        cx[:, o : o + 65],
                        exs[(hh, j)][:, l * P : (l + 1) * P],
                        vv[j][:, h, 0:65],
                        start=(j == 0), stop=(j == ST - 1),
                    )
            rc = rpl.tile([P, HPT], F32, tag="rc", name=f"rc{t}_{l}")
            dens = cx[:, 0 : 2 * 65].rearrange("p (h x) -> p h x", x=65)[:, :, 64:65]
            nc.vector.reciprocal(rc[:].rearrange("p (h x) -> p h x", x=1), dens)
            eng = norm_eng or nc.vector
            for hh in range(HPT):
                eng.scalar_tensor_tensor(
                    ot[:, l, hh * HD : (hh + 1) * HD],
                    cx[:, hh * 65 : hh * 65 + 64],
                    rc[:, hh : hh + 1],
                    bvb[:, (HPT * t + hh) * HD : (HPT * t + hh + 1) * HD],
                    ALU.mult,
                    ALU.add,
                )

        # ---- software-pipelined schedule ----
        # preamble: proj(0) unsmeared (Act idle anyway at start); K goes to
        # the big pool so it need not wait for Q's bias-add drain, and V'(0)
        # fills the PE while kt(0)'s bias-add drains.
        p0 = proj_steps(0, kpool=big)
        for _ in range(2 * KT + 1):
            next(p0)

        vgen = vprime_steps()
        next(vgen)
        nvp = 1  # V' tiles emitted so far

        ots = {}
        ctx_done = {}

        def ctx_unit(tc_, l, split_dma=False, norm_eng=None, cx_pool=None,
                     cx_tag="cx"):
            """Emit one ctx+normalize unit; DMA the o-tile column when all
            ST units of tc_ have been emitted (split_dma: one DMA per l)."""
            if tc_ not in ots:
                ots[tc_] = outp.tile([P, ST, P], F32, tag="ou", name=f"ou{tc_}")
                ctx_done[tc_] = 0
            emit_ctx(tc_, l, ots[tc_], norm_eng=norm_eng, cx_pool=cx_pool,
                     cx_tag=cx_tag)
            ctx_done[tc_] += 1
            if split_dma:
                if l % 2:  # DMA l-1..l as one transfer, alternating queues
                    eng = nc.scalar if l % 4 == 1 else nc.sync
                    eng.dma_start(
                        out=outD[
                            (l - 1) * P : (l + 1) * P,
                            tc_ * P : (tc_ + 1) * P,
                        ].rearrange("(l p) c -> p l c", p=P),
                        in_=ots[tc_][:, l - 1 : l + 1, :],
                    )
            elif ctx_done[tc_] == ST:
                nc.sync.dma_start(
                    out=outD[:, tc_ * P : (tc_ + 1) * P].rearrange(
                        "(l p) c -> p l c", p=P
                    ),
                    in_=ots[tc_][:],
                )

        # sections 0..NT-1: scores/exp(t), ctx(t-2), proj(t+1) smear, V' smear
        for t in range(NT):
            pgen = proj_steps(t + 1) if t + 1 < NT else None
            for j in range(ST):
                emit_scores_exp(t, j)
                # V' smear: 4 tiles in section 0 (after wv DMAs land), 4 in 1
                if nvp < ST and (t == 0 and j >= 3 or t == 1):
                    next(vgen)
                    nvp += 1
                if t >= 2:
                    ctx_unit(t - 2, j)
                if t == NT - 1 and j >= 2:
                    ctx_unit(NT - 2, j - 2)  # pull-in: no proj smear this section
                if pgen is not None:
                    next(pgen)  # 2 k-steps per j
                    next(pgen)
            if pgen is not None:
                next(pgen)  # flush the trailing bias-add
            load_w(t + 2)

        # tail: remaining ctx(NT-2) units, then ctx(NT-1) with per-l DMAs.
        # Only the drain chains remain: rotate the ctx PSUM through the
        # now-idle proj/scores rings (3 chains in flight) and alternate the
        # normalize between DVE and GPSIMD so two engines drain in parallel.
        tail_rot = [(cxp, "cx"), (prp, "pr"), (big, "big")]
        tail = [(NT - 2, ST - 2, False), (NT - 2, ST - 1, False)] + [
            (NT - 1, l, True) for l in range(ST)
        ]
        for i, (tc_, l, split) in enumerate(tail):
            pool_, tag_ = tail_rot[i % 3]
            # normalize must stay on DVE: GPSIMD cannot access PSUM on HW
            ctx_unit(tc_, l, split_dma=split, cx_pool=pool_, cx_tag=tag_)

    nc.compile()
    return nc


_CACHE = {}


def _get_program(S, H, NH, num_devices):
    key = (S, H, NH, num_devices)
    if key not in _CACHE:
        _CACHE[key] = build_program(S, H, NH, num_devices)
    return _CACHE[key]


def make_in_maps(hidden_states, Wq, bq, Wk, bk, Wv, bv):
    B, S, H = hidden_states.shape
    NT = H // P
    KT = H // P
    # wq/wk pre-tiled: row (t*P+p) holds concat over k of wT[k*P+p, t*P:(t+1)*P]
    def pack_w(W):
        wT = np.ascontiguousarray(W.T.astype(np.float32))  # [h, o]
        w4 = wT.reshape(KT, P, NT, P)                      # [k, p, t, c]
        return np.ascontiguousarray(
            w4.transpose(2, 1, 0, 3).reshape(NT * P, KT * P)
        ).astype(ml_dtypes.bfloat16)

    wqTp = pack_w(Wq)
    wkTp = pack_w(Wk)
    wvT = np.ascontiguousarray(Wv.T.astype(np.float32)).astype(ml_dtypes.bfloat16)
    bqk = np.ascontiguousarray(
        np.concatenate(
            [bq.reshape(NT, P).T, bk.reshape(NT, P).T], axis=1
        ).astype(np.float32)
    )
    bv_row = bv.astype(np.float32).reshape(1, H)
    in_maps = []
    for b in range(B):
        in_maps.append(
            {
                "hsT": np.ascontiguousarray(
                    hidden_states[b].T.astype(np.float32)
                ).astype(ml_dtypes.bfloat16),
                "wqTp": wqTp,
                "wkTp": wkTp,
                "wvT": wvT,
                "bqk": bqk,
                "bv_row": bv_row,
            }
        )
    return in_maps


def kernel(hidden_states, Wq, bq, Wk, bk, Wv, bv):
    hidden_states = np.asarray(hidden_states, dtype=np.float32)
    Wq = np.asarray(Wq, dtype=np.float32)
    bq = np.asarray(bq, dtype=np.float32)
    Wk = np.asarray(Wk, dtype=np.float32)
    bk = np.asarray(bk, dtype=np.float32)
    Wv = np.asarray(Wv, dtype=np.float32)
    bv = np.asarray(bv, dtype=np.float32)

    B, S, H = hidden_states.shape
    NH = H // HD
    assert B == N_CORES, "one batch element per core"

    nc = _get_program(S, H, NH, N_CORES)
    in_maps = make_in_maps(hidden_states, Wq, bq, Wk, bk, Wv, bv)
    res = run_bass_kernel_spmd(nc, in_maps, core_ids=list(range(N_CORES)))
    out = np.empty((B, S, H), np.float32)
    for b in range(B):
        out[b] = res.results[b]["out"]
    return out


if __name__ == "__main__":
    build_program(1024, 1024, 16)
    print("build ok")
